# revision 61
# baseline (speedup 1.0000x reference)
"""Trainium2 Bass kernel for a 3-layer LSTM recurrent encoder.

Contract: kernel(**inputs) takes FULL inputs (as produced by
setup_inputs()) and returns the FULL output [256, 128, 16] fp32.

Strategy: data-parallel over the batch (256 tracks -> 8 cores x 32),
with the three LSTM layers software-pipelined in a block wavefront
(block = TB steps): at slot s, layer l runs the recurrence for block
s-l while the projection (P0), the batched input-gate matmuls (P1b ->
DRAM gx), and the output projection (P2) for neighbouring blocks run
in the gaps. This keeps the PE continuously busy (HAM clock gate stays
at 2.4 GHz) and hides each layer's serial gate-math chain behind the
other layers' matmuls.

Layout: hidden-state ring buffers (one per layer boundary) are
[128, RINGB*TB*128] bf16 where column (q%RINGQ)*512 + k*128 + s*32 + b
(t = 4q+s) holds h[track b, h-dim k*128+p] for step t. Both the
recurrence stationary (t, k fixed -> 32 contiguous cols) and the P1b
stationary (q, k fixed -> 128 contiguous cols) are 2D slices, as
walrus requires for ldweights. The per-step recurrence is 4 concurrent
PE column-tile matmul groups (col group j computes gate chunk j =
[i|f|o|g] of h-tile j into PSUM partitions 32j..32j+32, accumulating
the identity-injected gx first, then the 4 Wh k-tiles), full-partition
[128, x] gate math, and a single 128x128 PE transpose per step.
All matmul operands are bf16 (1 cycle/row); PSUM and gate math fp32.
Gate columns are host-permuted per 128-block to [i|f|o|g]; the +1.0
forget bias is folded into b.
"""

import sys

sys.path.insert(0, "/opt/trn_rl_repo")

import ml_dtypes
import numpy as np

import concourse.bass as bass
import concourse.bacc as bacc
import concourse.mybir as mybir
import concourse.tile as tile
from concourse.bass_utils import run_bass_kernel_spmd
from concourse.masks import make_identity

AF = mybir.ActivationFunctionType
F32 = mybir.dt.float32
F32R = mybir.dt.float32r
BF16 = mybir.dt.bfloat16

B, T, F_IN = 256, 128, 16
H, L, OUT = 512, 3, 16
NCORES = 8
BL = B // NCORES          # 32 tracks per core
R = BL * T                # 4096 tokens per core
H4 = 4 * H                # 2048 gate columns
KT = H // 128             # 4 K-tiles of the hidden dim
NCH = H4 // 512           # 4 gate chunks of 512
TB = 8                    # wavefront block: steps per block
NBLK = T // TB            # 16 blocks
QB = TB // 4              # q-slots per block (4 steps per q)
RINGB = 3                 # ring capacity in blocks per boundary
RINGQ = RINGB * QB        # ring capacity in q-slots
LAG = 2                   # block lag between layers (P1b gets a slot of slack)
BF = ml_dtypes.bfloat16

_PROG = None


def _gate_perm():
    """Column permutation: per 128-block n -> [i_n, f_n, o_n, g_n]."""
    idx = []
    for n in range(KT):
        blk = np.arange(n * 128, (n + 1) * 128)
        idx.append(0 * H + blk)  # i
        idx.append(2 * H + blk)  # f
        idx.append(3 * H + blk)  # o
        idx.append(1 * H + blk)  # g
    return np.concatenate(idx)


def _ap(t, offset, dims):
    """AP over tile t at the given free offset with custom free dims."""
    a = t[:]
    return bass.AP(tensor=a.tensor, offset=a.offset + offset,
                   ap=[list(a.ap[0])] + dims)


def _qcol(t_step):
    """Ring column base (q-slot) for global step t_step."""
    q = t_step // 4
    return (q % RINGQ) * 512


def _build():
    nc = bacc.Bacc("TRN2", target_bir_lowering=False, debug=False,
                   num_devices=NCORES)

    xT_d = nc.dram_tensor("xT", [F_IN, R], BF16, kind="ExternalInput").ap()
    pw_d = nc.dram_tensor("pw", [F_IN, H], BF16, kind="ExternalInput").ap()
    pb_d = nc.dram_tensor("pb", [H, 1], F32, kind="ExternalInput").ap()
    wx_d = nc.dram_tensor("wx", [L, H, H4], BF16, kind="ExternalInput").ap()
    wh_d = nc.dram_tensor("wh", [L, H, H4], BF16, kind="ExternalInput").ap()
    bi_d = nc.dram_tensor("bi", [L, H4], BF16, kind="ExternalInput").ap()
    wo_d = nc.dram_tensor("wo", [H, OUT], BF16, kind="ExternalInput").ap()
    ob_d = nc.dram_tensor("ob", [OUT, 1], F32, kind="ExternalInput").ap()
    yT_d = nc.dram_tensor("yT", [OUT, R], F32, kind="ExternalOutput").ap()
    gx_d = nc.dram_tensor("gx", [L, R, H4], BF16, kind="Internal").ap()

    with tile.TileContext(nc) as tc:
        const = tc.alloc_tile_pool(name="const", bufs=1)
        idf = const.tile([32, 32], F32, tag="idf")
        make_identity(nc, idf)
        ident = const.tile([32, 32], BF16, tag="ident")
        nc.vector.tensor_copy(ident[:], idf[:])
        idf128 = const.tile([128, 128], F32, tag="idf128")
        make_identity(nc, idf128)
        id128 = const.tile([128, 128], BF16, tag="id128")
        nc.vector.tensor_copy(id128[:], idf128[:])

        # ---- persistent weights / inputs / biases / states ----
        wp = tc.alloc_tile_pool(name="wp", bufs=1)
        wh = [[wp.tile([128, H4], BF16, tag=f"wh{l}_{k}", name=f"wh{l}_{k}")
               for k in range(KT)] for l in range(L)]
        wx = [[wp.tile([128, H4], BF16, tag=f"wx{l}_{k}", name=f"wx{l}_{k}")
               for k in range(KT)] for l in range(L)]
        for l in range(L):
            for k in range(KT):
                nc.sync.dma_start(wh[l][k][:],
                                  wh_d[l, k * 128:(k + 1) * 128, :])
                nc.sync.dma_start(wx[l][k][:],
                                  wx_d[l, k * 128:(k + 1) * 128, :])
        bias = []
        for l in range(L):
            bt = wp.tile([128, H4], BF16, tag=f"bias{l}", name=f"bias{l}")
            nc.gpsimd.dma_start(
                bt[:], bass.AP(tensor=bi_d.tensor, offset=l * H4,
                               ap=[[0, 128], [1, H4]]))
            bias.append(bt)
        pwt = wp.tile([F_IN, H], BF16, tag="pwt")
        nc.sync.dma_start(pwt[:], pw_d)
        # proj bias as [128, KT]: column m holds pb[m*128 : (m+1)*128]
        pbt = wp.tile([128, KT], F32, tag="pbt")
        nc.sync.dma_start(
            pbt[:], bass.AP(tensor=pb_d.tensor, offset=0,
                            ap=[[1, 128], [128, KT]]))
        wo = [wp.tile([128, OUT], BF16, tag=f"wo{k}", name=f"wot{k}")
              for k in range(KT)]
        for k in range(KT):
            nc.sync.dma_start(wo[k][:], wo_d[k * 128:(k + 1) * 128, :])
        obt = wp.tile([OUT, 1], F32, tag="obt")
        nc.sync.dma_start(obt[:], ob_d)
        c_sb = [wp.tile([128, 128], F32, tag=f"c{l}", name=f"c{l}")
                for l in range(L)]
        for l in range(L):
            nc.vector.memset(c_sb[l], 0.0)

        # hidden-state rings: boundary r holds output of layer r-1
        # (r=0: xp from the projection)
        hp = tc.alloc_tile_pool(name="hp", bufs=1)
        HR = [hp.tile([128, RINGB * TB * 128], BF16, tag=f"HR{r}",
                      name=f"HR{r}") for r in range(L + 1)]

        ctxpools = [const, wp, hp]

        with tc.tile_pool(name="hrelu", bufs=2) as hrp, \
             tc.tile_pool(name="gxs", bufs=4) as gxs, \
             tc.tile_pool(name="gq", bufs=7) as gq, \
             tc.tile_pool(name="rp", bufs=3) as rp, \
             tc.tile_pool(name="xtb", bufs=2) as xtb, \
             tc.tile_pool(name="ys", bufs=2) as ysp, \
             tc.tile_pool(name="gps", bufs=3, space="PSUM") as gps, \
             tc.tile_pool(name="tps", bufs=2, space="PSUM") as tps, \
             tc.tile_pool(name="bps", bufs=1, space="PSUM") as bps:

            TOK = TB * 32          # tokens per block

            def p0_block(b):
                """Projection for block b -> HR[0] ring."""
                base = (b * QB % RINGQ) * 512
                xt = xtb.tile([F_IN, TOK], BF16)
                nc.sync.dma_start(xt[:], xT_d[:, b * TOK:(b + 1) * TOK])
                for m in range(KT):
                    ps = bps.tile([128, TOK], F32)
                    nc.tensor.matmul(ps[:], pwt[:, m * 128:(m + 1) * 128],
                                     xt[:], start=True, stop=True)
                    nc.scalar.activation(
                        _ap(HR[0], base + m * 128,
                            [[512, QB], [32, 4], [1, 32]]),
                        ps[:], AF.Relu, bias=pbt[:, m:m + 1])

            def p1b_half(l, b, mq, half, srcf):
                """gx for layer l, block b, q-slot mq, chunk pair `half`
                (8 matmuls, ~1.7us of PE work). k-outer over the n-pair
                so each ldweights feeds 2 matmuls."""
                src, soff = srcf()
                off = soff + mq * 512
                pss = [bps.tile([128, 512], F32, name=f"p1bps{p}")
                       for p in range(2)]
                for k in range(KT):
                    for p in range(2):
                        n = 2 * half + p
                        nc.tensor.matmul(
                            pss[p][:],
                            src[:, off + k * 128:off + (k + 1) * 128],
                            wx[l][k][:, n * 512:(n + 1) * 512],
                            start=(k == 0), stop=(k == KT - 1))
                for p in range(2):
                    n = 2 * half + p
                    g = gxs.tile([128, 512], BF16)
                    nc.vector.tensor_add(
                        g[:], pss[p][:],
                        bias[l][:, n * 512:(n + 1) * 512])
                    row = (b * QB + mq) * 128
                    nc.sync.dma_start(
                        gx_d[l, row:row + 128, n * 512:(n + 1) * 512],
                        g[:])

            def relu_block(l, b):
                """relu'd copy of HR[l] block b (P1b stationary source)."""
                base = (b * QB % RINGQ) * 512
                srcT = hrp.tile([128, TB * 128], BF16)
                nc.scalar.activation(
                    srcT[:], HR[l][:, base:base + TB * 128], AF.Relu)
                return srcT

            def p2_block(b):
                """Output projection for block b from HR[3] ring."""
                base = (b * QB % RINGQ) * 512
                ps = bps.tile([OUT, TOK], F32)
                for k in range(KT):
                    nc.tensor.matmul(
                        ps[:], wo[k][:],
                        _ap(HR[3], base + k * 128, [[512, QB], [1, 128]]),
                        start=(k == 0), stop=(k == KT - 1))
                y = ysp.tile([OUT, TOK], F32)
                nc.scalar.activation(y[:], ps[:], AF.Identity, bias=obt[:])
                nc.sync.dma_start(yT_d[:, b * TOK:(b + 1) * TOK], y[:])

            def load_img(l, t):
                """Prefetch step t's gx as a [128, 512] PSUM-layout image
                (partition 32j+b <- gx row t*32+b, cols j*512..)."""
                img = gq.tile([128, 512], BF16, name="gximg")
                nc.sync.dma_start(
                    img[:],
                    bass.AP(tensor=gx_d.tensor,
                            offset=l * R * H4 + t * 32 * H4,
                            ap=[[512, 4], [2048, 32], [1, 512]]))
                return img

            def step_mm(l, t, img, on_dve=False):
                """Matmul phase of one recurrence step: ACT (or DVE) copies
                the gx image into PSUM, then 4 col-tiled Wh k-rounds
                accumulate on top (start=False rides the bank's
                has_written bits)."""
                ps = gps.tile([128, 512], F32)
                if on_dve:
                    nc.vector.tensor_copy(ps[:], img[:])
                else:
                    nc.scalar.activation(ps[:], img[:], AF.Identity)
                if t > 0:
                    pbase = _qcol(t - 1) + ((t - 1) % 4) * 32
                    for k in range(KT):
                        stat = HR[l + 1][:, pbase + k * 128:
                                         pbase + k * 128 + 32]
                        for j in range(NCH):
                            nc.tensor.matmul(
                                ps[32 * j:32 * (j + 1), :], stat,
                                wh[l][k][:, j * 512:(j + 1) * 512],
                                start=False, stop=(k == KT - 1),
                                tile_position=(0, 32 * j))
                return ps

            def step_gate_a(l, ps):
                """First half of the gate math: sigmoid/tanh of the
                pre-activations and the cell-state update."""
                S = rp.tile([128, 384], F32)
                nc.scalar.activation(S[:], ps[:, 0:384], AF.Sigmoid)
                G = rp.tile([128, 128], F32)
                nc.scalar.activation(G[:], ps[:, 384:512], AF.Tanh)
                t1 = rp.tile([128, 128], F32)
                nc.vector.tensor_mul(t1[:], S[:, 0:128], G[:])
                t2 = rp.tile([128, 128], F32)
                nc.vector.tensor_mul(t2[:], S[:, 128:256], c_sb[l][:])
                nc.vector.tensor_add(c_sb[l][:], t1[:], t2[:])
                return S

            def step_gate_b(l, S):
                """Second half: h = sigmoid(o) * tanh(c)."""
                th = rp.tile([128, 128], F32)
                nc.scalar.activation(th[:], c_sb[l][:], AF.Tanh)
                h_all = rp.tile([128, 128], BF16)
                nc.vector.tensor_mul(h_all[:], S[:, 256:384], th[:])
                return h_all

            def step_tr(l, t, h_all):
                """Transpose h back into the layer's output ring."""
                tp = tps.tile([128, 128], BF16)
                nc.tensor.transpose(tp[:], h_all[:], id128[:])
                nc.vector.tensor_copy(
                    _ap(HR[l + 1], _qcol(t) + (t % 4) * 32,
                        [[128, 4], [1, 32]]),
                    tp[:])

            hr_src = {}

            def slot_units(s):
                """Work list of ~1.7us boundary pieces for slot s; dealt
                evenly across the slot's TB step-triples."""
                units = []
                if s + 1 < NBLK:
                    b0 = s + 1
                    units.append(lambda b=b0: p0_block(b))
                    for mq in range(QB):
                        for h in range(2):
                            units.append(
                                lambda b=b0, mq=mq, h=h: p1b_half(
                                    0, b, mq, h,
                                    lambda: (HR[0],
                                             (b * QB % RINGQ) * 512)))
                for l, db in ((1, -1), (2, -3)):
                    b = s + db
                    if 0 <= b < NBLK:
                        def mkrelu(l=l, b=b):
                            hr_src[l] = relu_block(l, b)
                        units.append(mkrelu)
                        for mq in range(QB):
                            for h in range(2):
                                units.append(
                                    lambda l=l, b=b, mq=mq, h=h: p1b_half(
                                        l, b, mq, h,
                                        lambda l=l: (hr_src[l], 0)))
                if 0 <= s - 5 < NBLK:
                    units.append(lambda b=s - 5: p2_block(b))
                return units

            # ---- prelude: xp + gx for the first block of layer 0 ----
            p0_block(0)
            for mq in range(QB):
                for h in range(2):
                    p1b_half(0, 0, mq, h, lambda: (HR[0], 0))

            # ---- wavefront (lag LAG blocks per layer) ----
            # Per step-triple: all active layers' matmuls first, then the
            # gate chains, then the transposes, with boundary pieces
            # emitted before the last transpose so the PE has work while
            # the last chain drains.
            img_cache = {}
            for s in range(NBLK + LAG * (L - 1) + 1):
                units = slot_units(s)
                nu = len(units)
                for i in range(TB):
                    act = [l for l in range(L) if 0 <= s - LAG * l < NBLK]
                    ts = {l: (s - LAG * l) * TB + i for l in act}
                    pss = {}
                    for l in act:
                        img = img_cache.pop((l, ts[l]), None)
                        if img is None:
                            img = load_img(l, ts[l])
                        pss[l] = step_mm(l, ts[l], img,
                                         on_dve=(l == act[0]))
                    # prefetch next step's gx image (within-block only, so
                    # the DMA never waits across a P1b that hasn't run)
                    for l in act:
                        tn = ts[l] + 1
                        if tn % TB != 0 and tn < T:
                            img_cache[(l, tn)] = load_img(l, tn)
                    ua = units[nu * (2 * i) // (2 * TB):
                               nu * (2 * i + 1) // (2 * TB)]
                    ub = units[nu * (2 * i + 1) // (2 * TB):
                               nu * (2 * i + 2) // (2 * TB)]
                    if len(act) == 3:
                        # software-pipeline the three gate chains so no
                        # engine FIFO head-of-line-blocks a later stage
                        l0, l1, l2 = act
                        S0 = step_gate_a(l0, pss[l0])
                        S1 = step_gate_a(l1, pss[l1])
                        h0 = step_gate_b(l0, S0)
                        for u in ua:
                            u()
                        S2 = step_gate_a(l2, pss[l2])
                        h1 = step_gate_b(l1, S1)
                        step_tr(l0, ts[l0], h0)
                        for u in ub:
                            u()
                        h2 = step_gate_b(l2, S2)
                        step_tr(l1, ts[l1], h1)
                        step_tr(l2, ts[l2], h2)
                    else:
                        Ss = {l: step_gate_a(l, pss[l]) for l in act}
                        for u in ua:
                            u()
                        hs = {l: step_gate_b(l, Ss[l]) for l in act}
                        for l in act[:-1]:
                            step_tr(l, ts[l], hs[l])
                        for u in ub:
                            u()
                        for l in act[-1:]:
                            step_tr(l, ts[l], hs[l])
                # cross-block prefetch: safe at slot end because the
                # producing P1b units for the next block were all emitted
                # above, so DMA queue FIFO order still respects the
                # write->read dependency
                for l in range(L):
                    bn = s + 1 - LAG * l
                    if 0 <= bn < NBLK:
                        img_cache[(l, bn * TB)] = load_img(l, bn * TB)

        for p in reversed(ctxpools):
            p.release()

    nc.compile()
    return nc


def _get_prog():
    global _PROG
    if _PROG is None:
        _PROG = _build()
    return _PROG


def _stage_inputs(x, proj_w, proj_b, lstm_w, lstm_b, out_w, out_b):
    perm = _gate_perm()
    lb = np.asarray(lstm_b, np.float32).copy()
    lb[:, 2 * H:3 * H] += 1.0          # forget-gate +1.0 folded into bias
    lw = np.asarray(lstm_w, np.float32)
    shared = {
        "pw": np.ascontiguousarray(np.asarray(proj_w, BF)),
        "pb": np.ascontiguousarray(np.asarray(proj_b, np.float32).reshape(H, 1)),
        "wx": np.ascontiguousarray(lw[:, :H, :][:, :, perm].astype(BF)),
        "wh": np.ascontiguousarray(lw[:, H:, :][:, :, perm].astype(BF)),
        "bi": np.ascontiguousarray(lb[:, perm].astype(BF)),
        "wo": np.ascontiguousarray(np.asarray(out_w, BF)),
        "ob": np.ascontiguousarray(np.asarray(out_b, np.float32).reshape(OUT, 1)),
    }
    x = np.asarray(x, np.float32)
    in_maps = []
    for c in range(NCORES):
        xs = x[c * BL:(c + 1) * BL]                     # [32, 128, 16]
        xT = np.ascontiguousarray(
            xs.transpose(2, 1, 0).reshape(F_IN, R).astype(BF))
        in_maps.append({"xT": xT, **shared})
    return in_maps


def kernel(x, proj_w, proj_b, lstm_w, lstm_b, out_w, out_b, _trace=False):
    nc = _get_prog()
    in_maps = _stage_inputs(x, proj_w, proj_b, lstm_w, lstm_b, out_w, out_b)
    res = run_bass_kernel_spmd(nc, in_maps, core_ids=list(range(NCORES)),
                               trace=_trace)
    y = np.empty((B, T, OUT), np.float32)
    for c in range(NCORES):
        yT = res.results[c]["yT"]                       # [16, 4096]
        y[c * BL:(c + 1) * BL] = yT.reshape(OUT, T, BL).transpose(2, 1, 0)
    kernel._last_results = res
    return y


# revision 65
# speedup vs baseline: 1.0990x; 1.0990x over previous
"""Trainium2 Bass kernel for a 3-layer LSTM recurrent encoder.

Contract: kernel(**inputs) takes FULL inputs (as produced by
setup_inputs()) and returns the FULL output [256, 128, 16] fp32.

Strategy: data-parallel over the batch (256 tracks -> 8 cores x 32),
with the three LSTM layers software-pipelined in a block wavefront
(block = TB steps): at slot s, layer l runs the recurrence for block
s-l while the projection (P0), the batched input-gate matmuls (P1b ->
DRAM gx), and the output projection (P2) for neighbouring blocks run
in the gaps. This keeps the PE continuously busy (HAM clock gate stays
at 2.4 GHz) and hides each layer's serial gate-math chain behind the
other layers' matmuls.

Layout: hidden-state ring buffers (one per layer boundary) are
[128, RINGB*TB*128] bf16 where column (q%RINGQ)*512 + k*128 + s*32 + b
(t = 4q+s) holds h[track b, h-dim k*128+p] for step t. Both the
recurrence stationary (t, k fixed -> 32 contiguous cols) and the P1b
stationary (q, k fixed -> 128 contiguous cols) are 2D slices, as
walrus requires for ldweights. The per-step recurrence is 4 concurrent
PE column-tile matmul groups (col group j computes gate chunk j =
[i|f|o|g] of h-tile j into PSUM partitions 32j..32j+32, accumulating
the identity-injected gx first, then the 4 Wh k-tiles), full-partition
[128, x] gate math, and a single 128x128 PE transpose per step.
All matmul operands are bf16 (1 cycle/row); PSUM and gate math fp32.
Gate columns are host-permuted per 128-block to [i|f|o|g]; the +1.0
forget bias is folded into b.
"""

import sys

sys.path.insert(0, "/opt/trn_rl_repo")

import ml_dtypes
import numpy as np

import concourse.bass as bass
import concourse.bacc as bacc
import concourse.mybir as mybir
import concourse.tile as tile
from concourse.bass_utils import run_bass_kernel_spmd
from concourse.masks import make_identity

AF = mybir.ActivationFunctionType
F32 = mybir.dt.float32
F32R = mybir.dt.float32r
BF16 = mybir.dt.bfloat16

B, T, F_IN = 256, 128, 16
H, L, OUT = 512, 3, 16
NCORES = 8
BL = B // NCORES          # 32 tracks per core
R = BL * T                # 4096 tokens per core
H4 = 4 * H                # 2048 gate columns
KT = H // 128             # 4 K-tiles of the hidden dim
NCH = H4 // 512           # 4 gate chunks of 512
TB = 8                    # wavefront block: steps per block
NBLK = T // TB            # 16 blocks
QB = TB // 4              # q-slots per block (4 steps per q)
RINGB = 3                 # ring capacity in blocks per boundary
RINGQ = RINGB * QB        # ring capacity in q-slots
LAG = 2                   # block lag between layers (P1b gets a slot of slack)
BF = ml_dtypes.bfloat16

_PROG = None


def _gate_perm():
    """Column permutation: per 128-block n -> [i_n, f_n, o_n, g_n]."""
    idx = []
    for n in range(KT):
        blk = np.arange(n * 128, (n + 1) * 128)
        idx.append(0 * H + blk)  # i
        idx.append(2 * H + blk)  # f
        idx.append(3 * H + blk)  # o
        idx.append(1 * H + blk)  # g
    return np.concatenate(idx)


def _ap(t, offset, dims):
    """AP over tile t at the given free offset with custom free dims."""
    a = t[:]
    return bass.AP(tensor=a.tensor, offset=a.offset + offset,
                   ap=[list(a.ap[0])] + dims)


def _qcol(t_step):
    """Ring column base (q-slot) for global step t_step."""
    q = t_step // 4
    return (q % RINGQ) * 512


def _build():
    nc = bacc.Bacc("TRN2", target_bir_lowering=False, debug=False,
                   num_devices=NCORES)

    xT_d = nc.dram_tensor("xT", [F_IN, R], BF16, kind="ExternalInput").ap()
    pw_d = nc.dram_tensor("pw", [F_IN, H], BF16, kind="ExternalInput").ap()
    pb_d = nc.dram_tensor("pb", [H, 1], F32, kind="ExternalInput").ap()
    wx_d = nc.dram_tensor("wx", [L, H, H4], BF16, kind="ExternalInput").ap()
    wh_d = nc.dram_tensor("wh", [L, H, H4], BF16, kind="ExternalInput").ap()
    bi_d = nc.dram_tensor("bi", [L, H4], BF16, kind="ExternalInput").ap()
    wo_d = nc.dram_tensor("wo", [H, OUT], BF16, kind="ExternalInput").ap()
    ob_d = nc.dram_tensor("ob", [OUT, 1], F32, kind="ExternalInput").ap()
    yT_d = nc.dram_tensor("yT", [OUT, R], F32, kind="ExternalOutput").ap()
    gx_d = nc.dram_tensor("gx", [L, R, H4], BF16, kind="Internal").ap()

    with tile.TileContext(nc) as tc:
        const = tc.alloc_tile_pool(name="const", bufs=1)
        idf = const.tile([32, 32], F32, tag="idf")
        make_identity(nc, idf)
        ident = const.tile([32, 32], BF16, tag="ident")
        nc.vector.tensor_copy(ident[:], idf[:])
        idf128 = const.tile([128, 128], F32, tag="idf128")
        make_identity(nc, idf128)
        id128 = const.tile([128, 128], BF16, tag="id128")
        nc.vector.tensor_copy(id128[:], idf128[:])

        # ---- persistent weights / inputs / biases / states ----
        wp = tc.alloc_tile_pool(name="wp", bufs=1)
        wh = [[wp.tile([128, H4], BF16, tag=f"wh{l}_{k}", name=f"wh{l}_{k}")
               for k in range(KT)] for l in range(L)]
        wx = [[wp.tile([128, H4], BF16, tag=f"wx{l}_{k}", name=f"wx{l}_{k}")
               for k in range(KT)] for l in range(L)]
        for l in range(L):
            for k in range(KT):
                nc.sync.dma_start(wh[l][k][:],
                                  wh_d[l, k * 128:(k + 1) * 128, :])
                nc.sync.dma_start(wx[l][k][:],
                                  wx_d[l, k * 128:(k + 1) * 128, :])
        bias = []
        for l in range(L):
            bt = wp.tile([128, H4], BF16, tag=f"bias{l}", name=f"bias{l}")
            nc.gpsimd.dma_start(
                bt[:], bass.AP(tensor=bi_d.tensor, offset=l * H4,
                               ap=[[0, 128], [1, H4]]))
            bias.append(bt)
        pwt = wp.tile([F_IN, H], BF16, tag="pwt")
        nc.sync.dma_start(pwt[:], pw_d)
        # proj bias as [128, KT]: column m holds pb[m*128 : (m+1)*128]
        pbt = wp.tile([128, KT], F32, tag="pbt")
        nc.sync.dma_start(
            pbt[:], bass.AP(tensor=pb_d.tensor, offset=0,
                            ap=[[1, 128], [128, KT]]))
        wo = [wp.tile([128, OUT], BF16, tag=f"wo{k}", name=f"wot{k}")
              for k in range(KT)]
        for k in range(KT):
            nc.sync.dma_start(wo[k][:], wo_d[k * 128:(k + 1) * 128, :])
        obt = wp.tile([OUT, 1], F32, tag="obt")
        nc.sync.dma_start(obt[:], ob_d)
        c_sb = [wp.tile([128, 128], F32, tag=f"c{l}", name=f"c{l}")
                for l in range(L)]
        for l in range(L):
            nc.vector.memset(c_sb[l], 0.0)

        # hidden-state rings: boundary r holds output of layer r-1
        # (r=0: xp from the projection)
        hp = tc.alloc_tile_pool(name="hp", bufs=1)
        HR = [hp.tile([128, RINGB * TB * 128], BF16, tag=f"HR{r}",
                      name=f"HR{r}") for r in range(L + 1)]

        ctxpools = [const, wp, hp]

        with tc.tile_pool(name="hrelu", bufs=2) as hrp, \
             tc.tile_pool(name="gxs", bufs=4) as gxs, \
             tc.tile_pool(name="gq", bufs=3) as gq, \
             tc.tile_pool(name="rp", bufs=3) as rp, \
             tc.tile_pool(name="xtb", bufs=2) as xtb, \
             tc.tile_pool(name="ys", bufs=2) as ysp, \
             tc.tile_pool(name="gps", bufs=3, space="PSUM") as gps, \
             tc.tile_pool(name="tps", bufs=2, space="PSUM") as tps, \
             tc.tile_pool(name="bps", bufs=1, space="PSUM") as bps:

            TOK = TB * 32          # tokens per block

            def p0_block(b):
                """Projection for block b -> HR[0] ring."""
                base = (b * QB % RINGQ) * 512
                xt = xtb.tile([F_IN, TOK], BF16)
                nc.sync.dma_start(xt[:], xT_d[:, b * TOK:(b + 1) * TOK])
                for m in range(KT):
                    ps = bps.tile([128, TOK], F32)
                    nc.tensor.matmul(ps[:], pwt[:, m * 128:(m + 1) * 128],
                                     xt[:], start=True, stop=True)
                    nc.scalar.activation(
                        _ap(HR[0], base + m * 128,
                            [[512, QB], [32, 4], [1, 32]]),
                        ps[:], AF.Relu, bias=pbt[:, m:m + 1])

            def p1b_half(l, b, mq, half, srcf):
                """gx for layer l, block b, q-slot mq, chunk pair `half`
                (8 matmuls, ~1.7us of PE work). k-outer over the n-pair
                so each ldweights feeds 2 matmuls."""
                src, soff = srcf()
                off = soff + mq * 512
                pss = [bps.tile([128, 512], F32, name=f"p1bps{p}")
                       for p in range(2)]
                for k in range(KT):
                    for p in range(2):
                        n = 2 * half + p
                        nc.tensor.matmul(
                            pss[p][:],
                            src[:, off + k * 128:off + (k + 1) * 128],
                            wx[l][k][:, n * 512:(n + 1) * 512],
                            start=(k == 0), stop=(k == KT - 1))
                for p in range(2):
                    n = 2 * half + p
                    g = gxs.tile([128, 512], BF16)
                    nc.vector.tensor_add(
                        g[:], pss[p][:],
                        bias[l][:, n * 512:(n + 1) * 512])
                    row = (b * QB + mq) * 128
                    nc.sync.dma_start(
                        gx_d[l, row:row + 128, n * 512:(n + 1) * 512],
                        g[:])

            def relu_block(l, b):
                """relu'd copy of HR[l] block b (P1b stationary source)."""
                base = (b * QB % RINGQ) * 512
                srcT = hrp.tile([128, TB * 128], BF16)
                nc.scalar.activation(
                    srcT[:], HR[l][:, base:base + TB * 128], AF.Relu)
                return srcT

            def p2_block(b):
                """Output projection for block b from HR[3] ring."""
                base = (b * QB % RINGQ) * 512
                ps = bps.tile([OUT, TOK], F32)
                for k in range(KT):
                    nc.tensor.matmul(
                        ps[:], wo[k][:],
                        _ap(HR[3], base + k * 128, [[512, QB], [1, 128]]),
                        start=(k == 0), stop=(k == KT - 1))
                y = ysp.tile([OUT, TOK], F32)
                nc.scalar.activation(y[:], ps[:], AF.Identity, bias=obt[:])
                nc.sync.dma_start(yT_d[:, b * TOK:(b + 1) * TOK], y[:])

            def load_img(l, t):
                """Prefetch step t's gx as a [128, 512] PSUM-layout image
                (partition 32j+b <- gx row t*32+b, cols j*512..)."""
                img = gq.tile([128, 512], BF16, name="gximg")
                nc.sync.dma_start(
                    img[:],
                    bass.AP(tensor=gx_d.tensor,
                            offset=l * R * H4 + t * 32 * H4,
                            ap=[[512, 4], [2048, 32], [1, 512]]))
                return img

            def step_mm(l, t, img):
                """Matmul phase of one recurrence step: ACT copies the gx
                image into PSUM, then 4 col-tiled Wh k-rounds accumulate
                on top (start=False rides the bank's has_written bits)."""
                ps = gps.tile([128, 512], F32)
                nc.scalar.activation(ps[:], img[:], AF.Identity)
                if t > 0:
                    pbase = _qcol(t - 1) + ((t - 1) % 4) * 32
                    for k in range(KT):
                        stat = HR[l + 1][:, pbase + k * 128:
                                         pbase + k * 128 + 32]
                        for j in range(NCH):
                            nc.tensor.matmul(
                                ps[32 * j:32 * (j + 1), :], stat,
                                wh[l][k][:, j * 512:(j + 1) * 512],
                                start=False, stop=(k == KT - 1),
                                tile_position=(0, 32 * j))
                return ps

            def step_gate_a(l, ps):
                """First half of the gate math: sigmoid/tanh of the
                pre-activations and the cell-state update."""
                S = rp.tile([128, 384], F32)
                nc.scalar.activation(S[:], ps[:, 0:384], AF.Sigmoid)
                G = rp.tile([128, 128], F32)
                nc.scalar.activation(G[:], ps[:, 384:512], AF.Tanh)
                t1 = rp.tile([128, 128], F32)
                nc.vector.tensor_mul(t1[:], S[:, 0:128], G[:])
                t2 = rp.tile([128, 128], F32)
                nc.vector.tensor_mul(t2[:], S[:, 128:256], c_sb[l][:])
                nc.vector.tensor_add(c_sb[l][:], t1[:], t2[:])
                return S

            def step_gate_b(l, S):
                """Second half: h = sigmoid(o) * tanh(c)."""
                th = rp.tile([128, 128], F32)
                nc.scalar.activation(th[:], c_sb[l][:], AF.Tanh)
                h_all = rp.tile([128, 128], BF16)
                nc.vector.tensor_mul(h_all[:], S[:, 256:384], th[:])
                return h_all

            def step_tr(l, t, h_all):
                """Transpose h back into the layer's output ring."""
                tp = tps.tile([128, 128], BF16)
                nc.tensor.transpose(tp[:], h_all[:], id128[:])
                nc.vector.tensor_copy(
                    _ap(HR[l + 1], _qcol(t) + (t % 4) * 32,
                        [[128, 4], [1, 32]]),
                    tp[:])

            hr_src = {}

            def slot_units(s):
                """Work list of ~1.7us boundary pieces for slot s; dealt
                evenly across the slot's TB step-triples."""
                units = []
                if s + 1 < NBLK:
                    b0 = s + 1
                    units.append(lambda b=b0: p0_block(b))
                    for mq in range(QB):
                        for h in range(2):
                            units.append(
                                lambda b=b0, mq=mq, h=h: p1b_half(
                                    0, b, mq, h,
                                    lambda: (HR[0],
                                             (b * QB % RINGQ) * 512)))
                for l, db in ((1, -1), (2, -3)):
                    b = s + db
                    if 0 <= b < NBLK:
                        def mkrelu(l=l, b=b):
                            hr_src[l] = relu_block(l, b)
                        units.append(mkrelu)
                        for mq in range(QB):
                            for h in range(2):
                                units.append(
                                    lambda l=l, b=b, mq=mq, h=h: p1b_half(
                                        l, b, mq, h,
                                        lambda l=l: (hr_src[l], 0)))
                if 0 <= s - 5 < NBLK:
                    units.append(lambda b=s - 5: p2_block(b))
                return units

            # ---- prelude: xp + gx for the first block of layer 0 ----
            p0_block(0)
            for mq in range(QB):
                for h in range(2):
                    p1b_half(0, 0, mq, h, lambda: (HR[0], 0))

            # ---- wavefront (lag LAG blocks per layer) ----
            # Per step-triple: all active layers' matmuls first, then the
            # gate chains, then the transposes, with boundary pieces
            # emitted before the last transpose so the PE has work while
            # the last chain drains.
            img_cache = {}
            for s in range(NBLK + LAG * (L - 1) + 1):
                units = slot_units(s)
                nu = len(units)
                for i in range(TB):
                    act = [l for l in range(L) if 0 <= s - LAG * l < NBLK]
                    ts = {l: (s - LAG * l) * TB + i for l in act}
                    pss = {}
                    for l in act:
                        img = img_cache.pop((l, ts[l]), None)
                        if img is None:
                            img = load_img(l, ts[l])
                        pss[l] = step_mm(l, ts[l], img)
                    # prefetch next step's gx image (within-block only, so
                    # the DMA never waits across a P1b that hasn't run)
                    for l in act:
                        tn = ts[l] + 1
                        if tn % TB != 0 and tn < T:
                            img_cache[(l, tn)] = load_img(l, tn)
                    ua = units[nu * (2 * i) // (2 * TB):
                               nu * (2 * i + 1) // (2 * TB)]
                    ub = units[nu * (2 * i + 1) // (2 * TB):
                               nu * (2 * i + 2) // (2 * TB)]
                    if len(act) == 3:
                        # software-pipeline the three gate chains so no
                        # engine FIFO head-of-line-blocks a later stage
                        l0, l1, l2 = act
                        S0 = step_gate_a(l0, pss[l0])
                        S1 = step_gate_a(l1, pss[l1])
                        h0 = step_gate_b(l0, S0)
                        S2 = step_gate_a(l2, pss[l2])
                        # boundary pieces go after a2/b2 so their DVE adds
                        # queue behind the chain-critical DVE ops
                        for u in ua:
                            u()
                        h1 = step_gate_b(l1, S1)
                        step_tr(l0, ts[l0], h0)
                        h2 = step_gate_b(l2, S2)
                        for u in ub:
                            u()
                        step_tr(l1, ts[l1], h1)
                        step_tr(l2, ts[l2], h2)
                    else:
                        Ss = {l: step_gate_a(l, pss[l]) for l in act}
                        for u in ua:
                            u()
                        hs = {l: step_gate_b(l, Ss[l]) for l in act}
                        for l in act[:-1]:
                            step_tr(l, ts[l], hs[l])
                        for u in ub:
                            u()
                        for l in act[-1:]:
                            step_tr(l, ts[l], hs[l])
                # cross-block prefetch: safe at slot end because the
                # producing P1b units for the next block were all emitted
                # above, so DMA queue FIFO order still respects the
                # write->read dependency
                for l in range(L):
                    bn = s + 1 - LAG * l
                    if 0 <= bn < NBLK:
                        img_cache[(l, bn * TB)] = load_img(l, bn * TB)

        for p in reversed(ctxpools):
            p.release()

    nc.compile()
    return nc


def _get_prog():
    global _PROG
    if _PROG is None:
        _PROG = _build()
    return _PROG


def _stage_inputs(x, proj_w, proj_b, lstm_w, lstm_b, out_w, out_b):
    perm = _gate_perm()
    lb = np.asarray(lstm_b, np.float32).copy()
    lb[:, 2 * H:3 * H] += 1.0          # forget-gate +1.0 folded into bias
    lw = np.asarray(lstm_w, np.float32)
    shared = {
        "pw": np.ascontiguousarray(np.asarray(proj_w, BF)),
        "pb": np.ascontiguousarray(np.asarray(proj_b, np.float32).reshape(H, 1)),
        "wx": np.ascontiguousarray(lw[:, :H, :][:, :, perm].astype(BF)),
        "wh": np.ascontiguousarray(lw[:, H:, :][:, :, perm].astype(BF)),
        "bi": np.ascontiguousarray(lb[:, perm].astype(BF)),
        "wo": np.ascontiguousarray(np.asarray(out_w, BF)),
        "ob": np.ascontiguousarray(np.asarray(out_b, np.float32).reshape(OUT, 1)),
    }
    x = np.asarray(x, np.float32)
    in_maps = []
    for c in range(NCORES):
        xs = x[c * BL:(c + 1) * BL]                     # [32, 128, 16]
        xT = np.ascontiguousarray(
            xs.transpose(2, 1, 0).reshape(F_IN, R).astype(BF))
        in_maps.append({"xT": xT, **shared})
    return in_maps


def kernel(x, proj_w, proj_b, lstm_w, lstm_b, out_w, out_b, _trace=False):
    nc = _get_prog()
    in_maps = _stage_inputs(x, proj_w, proj_b, lstm_w, lstm_b, out_w, out_b)
    res = run_bass_kernel_spmd(nc, in_maps, core_ids=list(range(NCORES)),
                               trace=_trace)
    y = np.empty((B, T, OUT), np.float32)
    for c in range(NCORES):
        yT = res.results[c]["yT"]                       # [16, 4096]
        y[c * BL:(c + 1) * BL] = yT.reshape(OUT, T, BL).transpose(2, 1, 0)
    kernel._last_results = res
    return y


# revision 66
# speedup vs baseline: 1.1059x; 1.0063x over previous
"""Trainium2 Bass kernel for a 3-layer LSTM recurrent encoder.

Contract: kernel(**inputs) takes FULL inputs (as produced by
setup_inputs()) and returns the FULL output [256, 128, 16] fp32.

Strategy: data-parallel over the batch (256 tracks -> 8 cores x 32),
with the three LSTM layers software-pipelined in a block wavefront
(block = TB steps): at slot s, layer l runs the recurrence for block
s-l while the projection (P0), the batched input-gate matmuls (P1b ->
DRAM gx), and the output projection (P2) for neighbouring blocks run
in the gaps. This keeps the PE continuously busy (HAM clock gate stays
at 2.4 GHz) and hides each layer's serial gate-math chain behind the
other layers' matmuls.

Layout: hidden-state ring buffers (one per layer boundary) are
[128, RINGB*TB*128] bf16 where column (q%RINGQ)*512 + k*128 + s*32 + b
(t = 4q+s) holds h[track b, h-dim k*128+p] for step t. Both the
recurrence stationary (t, k fixed -> 32 contiguous cols) and the P1b
stationary (q, k fixed -> 128 contiguous cols) are 2D slices, as
walrus requires for ldweights. The per-step recurrence is 4 concurrent
PE column-tile matmul groups (col group j computes gate chunk j =
[i|f|o|g] of h-tile j into PSUM partitions 32j..32j+32, accumulating
the identity-injected gx first, then the 4 Wh k-tiles), full-partition
[128, x] gate math, and a single 128x128 PE transpose per step.
All matmul operands are bf16 (1 cycle/row); PSUM and gate math fp32.
Gate columns are host-permuted per 128-block to [i|f|o|g]; the +1.0
forget bias is folded into b.
"""

import sys

sys.path.insert(0, "/opt/trn_rl_repo")

import ml_dtypes
import numpy as np

import concourse.bass as bass
import concourse.bacc as bacc
import concourse.mybir as mybir
import concourse.tile as tile
from concourse.bass_utils import run_bass_kernel_spmd
from concourse.masks import make_identity

AF = mybir.ActivationFunctionType
F32 = mybir.dt.float32
F32R = mybir.dt.float32r
BF16 = mybir.dt.bfloat16

B, T, F_IN = 256, 128, 16
H, L, OUT = 512, 3, 16
NCORES = 8
BL = B // NCORES          # 32 tracks per core
R = BL * T                # 4096 tokens per core
H4 = 4 * H                # 2048 gate columns
KT = H // 128             # 4 K-tiles of the hidden dim
NCH = H4 // 512           # 4 gate chunks of 512
TB = 8                    # wavefront block: steps per block
NBLK = T // TB            # 16 blocks
QB = TB // 4              # q-slots per block (4 steps per q)
RINGB = 3                 # ring capacity in blocks per boundary
RINGQ = RINGB * QB        # ring capacity in q-slots
LAG = 2                   # block lag between layers (P1b gets a slot of slack)
BF = ml_dtypes.bfloat16

_PROG = None


def _gate_perm():
    """Column permutation: per 128-block n -> [i_n, f_n, o_n, g_n]."""
    idx = []
    for n in range(KT):
        blk = np.arange(n * 128, (n + 1) * 128)
        idx.append(0 * H + blk)  # i
        idx.append(2 * H + blk)  # f
        idx.append(3 * H + blk)  # o
        idx.append(1 * H + blk)  # g
    return np.concatenate(idx)


def _ap(t, offset, dims):
    """AP over tile t at the given free offset with custom free dims."""
    a = t[:]
    return bass.AP(tensor=a.tensor, offset=a.offset + offset,
                   ap=[list(a.ap[0])] + dims)


def _qcol(t_step):
    """Ring column base (q-slot) for global step t_step."""
    q = t_step // 4
    return (q % RINGQ) * 512


def _build():
    nc = bacc.Bacc("TRN2", target_bir_lowering=False, debug=False,
                   num_devices=NCORES)

    xT_d = nc.dram_tensor("xT", [F_IN, R], BF16, kind="ExternalInput").ap()
    pw_d = nc.dram_tensor("pw", [F_IN, H], BF16, kind="ExternalInput").ap()
    pb_d = nc.dram_tensor("pb", [H, 1], F32, kind="ExternalInput").ap()
    wx_d = nc.dram_tensor("wx", [L, H, H4], BF16, kind="ExternalInput").ap()
    wh_d = nc.dram_tensor("wh", [L, H, H4], BF16, kind="ExternalInput").ap()
    bi_d = nc.dram_tensor("bi", [L, H4], BF16, kind="ExternalInput").ap()
    wo_d = nc.dram_tensor("wo", [H, OUT], BF16, kind="ExternalInput").ap()
    ob_d = nc.dram_tensor("ob", [OUT, 1], F32, kind="ExternalInput").ap()
    yT_d = nc.dram_tensor("yT", [OUT, R], F32, kind="ExternalOutput").ap()
    gx_d = nc.dram_tensor("gx", [L, R, H4], BF16, kind="Internal").ap()

    with tile.TileContext(nc) as tc:
        const = tc.alloc_tile_pool(name="const", bufs=1)
        idf = const.tile([32, 32], F32, tag="idf")
        make_identity(nc, idf)
        ident = const.tile([32, 32], BF16, tag="ident")
        nc.vector.tensor_copy(ident[:], idf[:])
        idf128 = const.tile([128, 128], F32, tag="idf128")
        make_identity(nc, idf128)
        id128 = const.tile([128, 128], BF16, tag="id128")
        nc.vector.tensor_copy(id128[:], idf128[:])

        # ---- persistent weights / inputs / biases / states ----
        wp = tc.alloc_tile_pool(name="wp", bufs=1)
        wh = [[wp.tile([128, H4], BF16, tag=f"wh{l}_{k}", name=f"wh{l}_{k}")
               for k in range(KT)] for l in range(L)]
        wx = [[wp.tile([128, H4], BF16, tag=f"wx{l}_{k}", name=f"wx{l}_{k}")
               for k in range(KT)] for l in range(L)]
        for l in range(L):
            for k in range(KT):
                nc.sync.dma_start(wh[l][k][:],
                                  wh_d[l, k * 128:(k + 1) * 128, :])
                nc.sync.dma_start(wx[l][k][:],
                                  wx_d[l, k * 128:(k + 1) * 128, :])
        bias = []
        for l in range(L):
            bt = wp.tile([128, H4], BF16, tag=f"bias{l}", name=f"bias{l}")
            nc.gpsimd.dma_start(
                bt[:], bass.AP(tensor=bi_d.tensor, offset=l * H4,
                               ap=[[0, 128], [1, H4]]))
            bias.append(bt)
        pwt = wp.tile([F_IN, H], BF16, tag="pwt")
        nc.sync.dma_start(pwt[:], pw_d)
        # proj bias as [128, KT]: column m holds pb[m*128 : (m+1)*128]
        pbt = wp.tile([128, KT], F32, tag="pbt")
        nc.sync.dma_start(
            pbt[:], bass.AP(tensor=pb_d.tensor, offset=0,
                            ap=[[1, 128], [128, KT]]))
        wo = [wp.tile([128, OUT], BF16, tag=f"wo{k}", name=f"wot{k}")
              for k in range(KT)]
        for k in range(KT):
            nc.sync.dma_start(wo[k][:], wo_d[k * 128:(k + 1) * 128, :])
        obt = wp.tile([OUT, 1], F32, tag="obt")
        nc.sync.dma_start(obt[:], ob_d)
        c_sb = [wp.tile([128, 128], F32, tag=f"c{l}", name=f"c{l}")
                for l in range(L)]
        for l in range(L):
            nc.vector.memset(c_sb[l], 0.0)

        # hidden-state rings: boundary r holds output of layer r-1
        # (r=0: xp from the projection)
        hp = tc.alloc_tile_pool(name="hp", bufs=1)
        HR = [hp.tile([128, RINGB * TB * 128], BF16, tag=f"HR{r}",
                      name=f"HR{r}") for r in range(L + 1)]

        ctxpools = [const, wp, hp]

        with tc.tile_pool(name="hrelu", bufs=2) as hrp, \
             tc.tile_pool(name="gxs", bufs=4) as gxs, \
             tc.tile_pool(name="gq", bufs=3) as gq, \
             tc.tile_pool(name="rp", bufs=3) as rp, \
             tc.tile_pool(name="xtb", bufs=2) as xtb, \
             tc.tile_pool(name="ys", bufs=2) as ysp, \
             tc.tile_pool(name="gps", bufs=3, space="PSUM") as gps, \
             tc.tile_pool(name="tps", bufs=2, space="PSUM") as tps, \
             tc.tile_pool(name="bps", bufs=1, space="PSUM") as bps:

            TOK = TB * 32          # tokens per block

            def p0_block(b):
                """Projection for block b -> HR[0] ring."""
                base = (b * QB % RINGQ) * 512
                xt = xtb.tile([F_IN, TOK], BF16)
                nc.sync.dma_start(xt[:], xT_d[:, b * TOK:(b + 1) * TOK])
                for m in range(KT):
                    ps = bps.tile([128, TOK], F32)
                    nc.tensor.matmul(ps[:], pwt[:, m * 128:(m + 1) * 128],
                                     xt[:], start=True, stop=True)
                    nc.scalar.activation(
                        _ap(HR[0], base + m * 128,
                            [[512, QB], [32, 4], [1, 32]]),
                        ps[:], AF.Relu, bias=pbt[:, m:m + 1])

            def p1b_half(l, b, mq, half, srcf):
                """gx for layer l, block b, q-slot mq, chunk pair `half`
                (8 matmuls, ~1.7us of PE work). k-outer over the n-pair
                so each ldweights feeds 2 matmuls."""
                src, soff = srcf()
                off = soff + mq * 512
                pss = [bps.tile([128, 512], F32, name=f"p1bps{p}")
                       for p in range(2)]
                for k in range(KT):
                    for p in range(2):
                        n = 2 * half + p
                        nc.tensor.matmul(
                            pss[p][:],
                            src[:, off + k * 128:off + (k + 1) * 128],
                            wx[l][k][:, n * 512:(n + 1) * 512],
                            start=(k == 0), stop=(k == KT - 1))
                for p in range(2):
                    n = 2 * half + p
                    g = gxs.tile([128, 512], BF16)
                    nc.vector.tensor_add(
                        g[:], pss[p][:],
                        bias[l][:, n * 512:(n + 1) * 512])
                    row = (b * QB + mq) * 128
                    nc.sync.dma_start(
                        gx_d[l, row:row + 128, n * 512:(n + 1) * 512],
                        g[:])

            def relu_block(l, b):
                """relu'd copy of HR[l] block b (P1b stationary source).
                On DVE (max with 0) to stay off the gate-math ACT FIFO."""
                base = (b * QB % RINGQ) * 512
                srcT = hrp.tile([128, TB * 128], BF16)
                nc.vector.tensor_scalar_max(
                    srcT[:], HR[l][:, base:base + TB * 128], 0.0)
                return srcT

            def p2_block(b):
                """Output projection for block b from HR[3] ring."""
                base = (b * QB % RINGQ) * 512
                ps = bps.tile([OUT, TOK], F32)
                for k in range(KT):
                    nc.tensor.matmul(
                        ps[:], wo[k][:],
                        _ap(HR[3], base + k * 128, [[512, QB], [1, 128]]),
                        start=(k == 0), stop=(k == KT - 1))
                y = ysp.tile([OUT, TOK], F32)
                nc.scalar.activation(y[:], ps[:], AF.Identity, bias=obt[:])
                nc.sync.dma_start(yT_d[:, b * TOK:(b + 1) * TOK], y[:])

            def load_img(l, t):
                """Prefetch step t's gx as a [128, 512] PSUM-layout image
                (partition 32j+b <- gx row t*32+b, cols j*512..)."""
                img = gq.tile([128, 512], BF16, name="gximg")
                nc.sync.dma_start(
                    img[:],
                    bass.AP(tensor=gx_d.tensor,
                            offset=l * R * H4 + t * 32 * H4,
                            ap=[[512, 4], [2048, 32], [1, 512]]))
                return img

            def step_mm(l, t, img):
                """Matmul phase of one recurrence step: ACT copies the gx
                image into PSUM, then 4 col-tiled Wh k-rounds accumulate
                on top (start=False rides the bank's has_written bits)."""
                ps = gps.tile([128, 512], F32)
                nc.scalar.activation(ps[:], img[:], AF.Identity)
                if t > 0:
                    pbase = _qcol(t - 1) + ((t - 1) % 4) * 32
                    for k in range(KT):
                        stat = HR[l + 1][:, pbase + k * 128:
                                         pbase + k * 128 + 32]
                        for j in range(NCH):
                            nc.tensor.matmul(
                                ps[32 * j:32 * (j + 1), :], stat,
                                wh[l][k][:, j * 512:(j + 1) * 512],
                                start=False, stop=(k == KT - 1),
                                tile_position=(0, 32 * j))
                return ps

            def step_gate_a(l, ps):
                """First half of the gate math: sigmoid/tanh of the
                pre-activations and the cell-state update."""
                S = rp.tile([128, 384], F32)
                nc.scalar.activation(S[:], ps[:, 0:384], AF.Sigmoid)
                G = rp.tile([128, 128], F32)
                nc.scalar.activation(G[:], ps[:, 384:512], AF.Tanh)
                t1 = rp.tile([128, 128], F32)
                nc.vector.tensor_mul(t1[:], S[:, 0:128], G[:])
                t2 = rp.tile([128, 128], F32)
                nc.vector.tensor_mul(t2[:], S[:, 128:256], c_sb[l][:])
                nc.vector.tensor_add(c_sb[l][:], t1[:], t2[:])
                return S

            def step_gate_b(l, S):
                """Second half: h = sigmoid(o) * tanh(c)."""
                th = rp.tile([128, 128], F32)
                nc.scalar.activation(th[:], c_sb[l][:], AF.Tanh)
                h_all = rp.tile([128, 128], BF16)
                nc.vector.tensor_mul(h_all[:], S[:, 256:384], th[:])
                return h_all

            def step_tr(l, t, h_all):
                """Transpose h back into the layer's output ring."""
                tp = tps.tile([128, 128], BF16)
                nc.tensor.transpose(tp[:], h_all[:], id128[:])
                nc.vector.tensor_copy(
                    _ap(HR[l + 1], _qcol(t) + (t % 4) * 32,
                        [[128, 4], [1, 32]]),
                    tp[:])

            hr_src = {}

            def slot_units(s):
                """Work list of ~1.7us boundary pieces for slot s; dealt
                evenly across the slot's TB step-triples."""
                units = []
                if s + 1 < NBLK:
                    b0 = s + 1
                    units.append(lambda b=b0: p0_block(b))
                    for mq in range(QB):
                        for h in range(2):
                            units.append(
                                lambda b=b0, mq=mq, h=h: p1b_half(
                                    0, b, mq, h,
                                    lambda: (HR[0],
                                             (b * QB % RINGQ) * 512)))
                for l, db in ((1, -1), (2, -3)):
                    b = s + db
                    if 0 <= b < NBLK:
                        def mkrelu(l=l, b=b):
                            hr_src[l] = relu_block(l, b)
                        units.append(mkrelu)
                        for mq in range(QB):
                            for h in range(2):
                                units.append(
                                    lambda l=l, b=b, mq=mq, h=h: p1b_half(
                                        l, b, mq, h,
                                        lambda l=l: (hr_src[l], 0)))
                if 0 <= s - 5 < NBLK:
                    units.append(lambda b=s - 5: p2_block(b))
                return units

            # ---- prelude: xp + gx for the first block of layer 0 ----
            p0_block(0)
            for mq in range(QB):
                for h in range(2):
                    p1b_half(0, 0, mq, h, lambda: (HR[0], 0))

            # ---- wavefront (lag LAG blocks per layer) ----
            # Per step-triple: all active layers' matmuls first, then the
            # gate chains, then the transposes, with boundary pieces
            # emitted before the last transpose so the PE has work while
            # the last chain drains.
            img_cache = {}
            for s in range(NBLK + LAG * (L - 1) + 1):
                units = slot_units(s)
                nu = len(units)
                for i in range(TB):
                    act = [l for l in range(L) if 0 <= s - LAG * l < NBLK]
                    ts = {l: (s - LAG * l) * TB + i for l in act}
                    pss = {}
                    for l in act:
                        img = img_cache.pop((l, ts[l]), None)
                        if img is None:
                            img = load_img(l, ts[l])
                        pss[l] = step_mm(l, ts[l], img)
                    # prefetch next step's gx image (within-block only, so
                    # the DMA never waits across a P1b that hasn't run)
                    for l in act:
                        tn = ts[l] + 1
                        if tn % TB != 0 and tn < T:
                            img_cache[(l, tn)] = load_img(l, tn)
                    ua = units[nu * (2 * i) // (2 * TB):
                               nu * (2 * i + 1) // (2 * TB)]
                    ub = units[nu * (2 * i + 1) // (2 * TB):
                               nu * (2 * i + 2) // (2 * TB)]
                    if len(act) == 3:
                        # software-pipeline the three gate chains so no
                        # engine FIFO head-of-line-blocks a later stage
                        l0, l1, l2 = act
                        S0 = step_gate_a(l0, pss[l0])
                        S1 = step_gate_a(l1, pss[l1])
                        h0 = step_gate_b(l0, S0)
                        S2 = step_gate_a(l2, pss[l2])
                        # boundary pieces go after a2/b2 so their DVE adds
                        # queue behind the chain-critical DVE ops
                        for u in ua:
                            u()
                        h1 = step_gate_b(l1, S1)
                        step_tr(l0, ts[l0], h0)
                        h2 = step_gate_b(l2, S2)
                        for u in ub:
                            u()
                        step_tr(l1, ts[l1], h1)
                        step_tr(l2, ts[l2], h2)
                    else:
                        Ss = {l: step_gate_a(l, pss[l]) for l in act}
                        for u in ua:
                            u()
                        hs = {l: step_gate_b(l, Ss[l]) for l in act}
                        for l in act[:-1]:
                            step_tr(l, ts[l], hs[l])
                        for u in ub:
                            u()
                        for l in act[-1:]:
                            step_tr(l, ts[l], hs[l])
                # cross-block prefetch: safe at slot end because the
                # producing P1b units for the next block were all emitted
                # above, so DMA queue FIFO order still respects the
                # write->read dependency
                for l in range(L):
                    bn = s + 1 - LAG * l
                    if 0 <= bn < NBLK:
                        img_cache[(l, bn * TB)] = load_img(l, bn * TB)

        for p in reversed(ctxpools):
            p.release()

    nc.compile()
    return nc


def _get_prog():
    global _PROG
    if _PROG is None:
        _PROG = _build()
    return _PROG


def _stage_inputs(x, proj_w, proj_b, lstm_w, lstm_b, out_w, out_b):
    perm = _gate_perm()
    lb = np.asarray(lstm_b, np.float32).copy()
    lb[:, 2 * H:3 * H] += 1.0          # forget-gate +1.0 folded into bias
    lw = np.asarray(lstm_w, np.float32)
    shared = {
        "pw": np.ascontiguousarray(np.asarray(proj_w, BF)),
        "pb": np.ascontiguousarray(np.asarray(proj_b, np.float32).reshape(H, 1)),
        "wx": np.ascontiguousarray(lw[:, :H, :][:, :, perm].astype(BF)),
        "wh": np.ascontiguousarray(lw[:, H:, :][:, :, perm].astype(BF)),
        "bi": np.ascontiguousarray(lb[:, perm].astype(BF)),
        "wo": np.ascontiguousarray(np.asarray(out_w, BF)),
        "ob": np.ascontiguousarray(np.asarray(out_b, np.float32).reshape(OUT, 1)),
    }
    x = np.asarray(x, np.float32)
    in_maps = []
    for c in range(NCORES):
        xs = x[c * BL:(c + 1) * BL]                     # [32, 128, 16]
        xT = np.ascontiguousarray(
            xs.transpose(2, 1, 0).reshape(F_IN, R).astype(BF))
        in_maps.append({"xT": xT, **shared})
    return in_maps


def kernel(x, proj_w, proj_b, lstm_w, lstm_b, out_w, out_b, _trace=False):
    nc = _get_prog()
    in_maps = _stage_inputs(x, proj_w, proj_b, lstm_w, lstm_b, out_w, out_b)
    res = run_bass_kernel_spmd(nc, in_maps, core_ids=list(range(NCORES)),
                               trace=_trace)
    y = np.empty((B, T, OUT), np.float32)
    for c in range(NCORES):
        yT = res.results[c]["yT"]                       # [16, 4096]
        y[c * BL:(c + 1) * BL] = yT.reshape(OUT, T, BL).transpose(2, 1, 0)
    kernel._last_results = res
    return y
